# revision 1
# baseline (speedup 1.0000x reference)
"""CRF forward-algorithm (log partition) kernel for 8 Trainium2 NeuronCores.

Strategy: segment-spliced exp-space scan.

The reference recurrence  fv' = logsumexp_prev(fv + T) + feat  is, in exp
space, a linear matvec chain  v' = diag(e_t) @ M @ v  with M = exp(T) fixed.
We split the T=16384 steps into S=1024 segments of L=16 and run all segments
in parallel from a guess vector, batched 129 columns per core so the PE array
runs dense [128x128] x [128x129] matmuls (full utilization) instead of
matvecs.  Products of positive matrices contract exponentially toward rank-1
(Perron-Frobenius), so the true correction at each segment junction is a pure
scalar kappa, measured exactly by re-running only the first D=8 steps of each
segment from the previous segment's endpoint (phase 2, also fully parallel —
logsumexp commutes with additive constants).  alpha = lse(final) + sum(kappa).

Per-step rescaling is folded into the emissions as a constant e^-8 (zero
cost); all bookkeeping scales are recovered analytically at the end.

Each core is fully independent (no collectives): core c owns segments
[c*128, c*128+128] (129 columns, one redundant boundary column so junction
sources are always core-local).  The host does the tiny O(S*N) final
assembly (kappa extraction + terminal logsumexp) in fp64.
"""

import numpy as np
import ml_dtypes

import concourse.bass as bass
import concourse.bacc as bacc
import concourse.mybir as mybir
import concourse.tile as tile

BF16_NP = ml_dtypes.bfloat16
BF16 = mybir.dt.bfloat16
F32 = mybir.dt.float32

SEQ_LEN = 16384
N_TAGS = 1024
START_IDX = 1022
STOP_IDX = 1023
NB = 8                 # 1024 tags = 8 blocks of 128 partitions
L = 16                 # segment length (steps)
D = 8                  # junction fixup depth (steps)
S = SEQ_LEN // L       # 1024 segments
NCORES = 8
BPC = S // NCORES      # 128 segments owned per core
NCOLS = BPC + 1        # 129 phase-1 columns (1 redundant boundary col)
CSCALE = 8.0           # constant per-step rescale folded into emissions

_CACHE = {}


def _build_program():
    nc = bacc.Bacc("TRN2", target_bir_lowering=False, debug=False)
    mt = nc.dram_tensor("mt", [N_TAGS, N_TAGS], BF16, kind="ExternalInput")
    vinit = nc.dram_tensor("vinit", [N_TAGS, NCOLS], BF16, kind="ExternalInput")
    e1 = nc.dram_tensor("e1", [L, 128, NB * NCOLS], BF16, kind="ExternalInput")
    e2 = nc.dram_tensor("e2", [D, 128, NB * BPC], BF16, kind="ExternalInput")
    snap = nc.dram_tensor("snap", [NB, 128, NCOLS], BF16, kind="ExternalOutput")
    yend = nc.dram_tensor("yend", [NB, 128, NCOLS], BF16, kind="ExternalOutput")
    zout = nc.dram_tensor("zout", [NB, 128, BPC], BF16, kind="ExternalOutput")

    with tile.TileContext(nc) as tc:
        with (
            tc.tile_pool(name="mpool", bufs=1) as mpool,
            tc.tile_pool(name="vpool", bufs=2) as vpool,
            tc.tile_pool(name="epool", bufs=3) as epool,
            tc.tile_pool(name="pspool", bufs=1, space="PSUM") as pspool,
        ):
            # Stationary operand: mt[prev, next]; section kb holds rows
            # [kb*128, kb*128+128) across all next-tags.
            mt_sb = mpool.tile([128, NB * N_TAGS], BF16)
            for kb in range(NB):
                nc.sync.dma_start(
                    mt_sb[:, kb * N_TAGS:(kb + 1) * N_TAGS],
                    mt[kb * 128:(kb + 1) * 128, :],
                )

            v_tiles = []
            for kb in range(NB):
                vt = vpool.tile([128, NCOLS], BF16, tag=f"v{kb}")
                nc.sync.dma_start(vt[:], vinit[kb * 128:(kb + 1) * 128, :])
                v_tiles.append(vt)

            def step(v_aps, e_row, ncols, out_dram=None):
                et = epool.tile([128, NB * ncols], BF16, tag="e")
                nc.sync.dma_start(et[:], e_row)
                new_tiles = []
                for mb in range(NB):
                    ps = pspool.tile([128, ncols], F32, tag=f"ps{mb}")
                    for kb in range(NB):
                        sec = kb * N_TAGS + mb * 128
                        nc.tensor.matmul(
                            ps[:],
                            mt_sb[:, sec:sec + 128],
                            v_aps[kb],
                            start=(kb == 0),
                            stop=(kb == NB - 1),
                        )
                    nv = vpool.tile([128, ncols], BF16, tag=f"v{mb}")
                    nc.vector.tensor_mul(
                        nv[:], ps[:], et[:, mb * ncols:(mb + 1) * ncols]
                    )
                    if out_dram is not None:
                        nc.sync.dma_start(out_dram[mb], nv[:])
                    new_tiles.append(nv)
                return new_tiles

            for s in range(L):
                out_d = snap if s + 1 == D else (yend if s + 1 == L else None)
                v_tiles = step([vt[:] for vt in v_tiles], e1[s], NCOLS, out_d)

            v_aps = [vt[:, 0:BPC] for vt in v_tiles]
            for s in range(D):
                out_d = zout if s + 1 == D else None
                new = step(v_aps, e2[s], BPC, out_d)
                v_aps = [vt[:] for vt in new]

    nc.compile()
    return nc


def _prepare_core_inputs(E, Mt_bf, vinit_base):
    """Per-core input dicts. E: [T, N] bf16 emissions exp(decoded - CSCALE)."""
    in_maps = []
    steps1 = np.arange(L)
    steps2 = np.arange(D)
    for c in range(NCORES):
        segs1 = np.minimum(c * BPC + np.arange(NCOLS), S - 1)
        segs2 = np.minimum(c * BPC + 1 + np.arange(BPC), S - 1)
        t1 = segs1 * L  # [NCOLS]
        t2 = segs2 * L  # [BPC]
        # a1[s, col, tag] -> e1[s, p, mb*NCOLS + col]
        a1 = E[t1[None, :] + steps1[:, None]]          # [L, NCOLS, N]
        a1 = a1.reshape(L, NCOLS, NB, 128)
        e1 = np.ascontiguousarray(a1.transpose(0, 3, 2, 1)).reshape(L, 128, NB * NCOLS)
        a2 = E[t2[None, :] + steps2[:, None]]          # [D, BPC, N]
        a2 = a2.reshape(D, BPC, NB, 128)
        e2 = np.ascontiguousarray(a2.transpose(0, 3, 2, 1)).reshape(D, 128, NB * BPC)
        vin = vinit_base.copy()
        if c == 0:
            vin[:, 0] = BF16_NP(0.0)
            vin[START_IDX, 0] = BF16_NP(1.0)
        in_maps.append({"mt": Mt_bf, "vinit": vin, "e1": e1, "e2": e2})
    return in_maps


def _assemble(transitions, results):
    """Host-side kappa extraction + terminal logsumexp (tiny, fp64)."""
    kappa_sum = 0.0
    max_spread = 0.0
    for c in range(NCORES):
        snap = results[c]["snap"].astype(np.float64)  # [NB, 128, NCOLS]
        zout = results[c]["zout"].astype(np.float64)  # [NB, 128, BPC]
        # col j of zout: junction for segment c*BPC+j+1; compare with snap col j+1
        nj = BPC if c < NCORES - 1 else BPC - 1  # core 7's last junction is dummy
        z = zout.reshape(N_TAGS, BPC)[:, :nj]
        sn = snap.reshape(N_TAGS, NCOLS)[:, 1:nj + 1]
        valid = (z > 0) & (sn > 0)
        with np.errstate(divide="ignore", invalid="ignore"):
            dlt = np.where(valid, np.log(z) - np.log(sn), np.nan)
        kap = np.nanmedian(dlt, axis=0)
        spread = np.nanmax(dlt, axis=0) - np.nanmin(dlt, axis=0)
        max_spread = max(max_spread, float(spread.max()))
        kappa_sum += float(kap.sum())

    y_last = results[NCORES - 1]["yend"].astype(np.float64).reshape(N_TAGS, NCOLS)[:, BPC - 1]
    with np.errstate(divide="ignore"):
        logx = np.log(y_last) + kappa_sum + CSCALE * SEQ_LEN
    term = logx + transitions[STOP_IDX].astype(np.float64)
    term = term[np.isfinite(term)]
    mx = term.max()
    alpha = mx + np.log(np.exp(term - mx).sum())
    return alpha, max_spread


def kernel(decoded, transitions, raw_outputs=None, outputs=None, _backend="hw"):
    decoded = np.asarray(decoded, dtype=np.float32)
    transitions = np.asarray(transitions, dtype=np.float32)

    Mt_bf = np.exp(transitions.astype(np.float64)).T.astype(BF16_NP)  # [prev, next]
    Mt_bf = np.ascontiguousarray(Mt_bf)
    E = np.exp(decoded - np.float32(CSCALE)).astype(BF16_NP)          # [T, N]
    vinit_base = np.full((N_TAGS, NCOLS), 1.0 / N_TAGS, dtype=BF16_NP)

    in_maps = _prepare_core_inputs(E, Mt_bf, vinit_base)

    if "nc" not in _CACHE:
        _CACHE["nc"] = _build_program()
    nc = _CACHE["nc"]

    if _backend == "sim":
        from concourse.bass_interp import CoreSim
        results = []
        for c in range(NCORES):
            sim = CoreSim(nc, trace=False)
            for k, v in in_maps[c].items():
                sim.tensor(k)[:] = v
            sim.simulate()
            results.append({k: np.array(sim.tensor(k)) for k in ("snap", "yend", "zout")})
    else:
        from concourse.bass_utils import run_bass_kernel_spmd
        res = run_bass_kernel_spmd(nc, in_maps, list(range(NCORES)))
        results = res.results

    alpha, max_spread = _assemble(transitions, results)
    if max_spread > 0.2:
        import sys
        print(f"kernel: WARNING junction spread {max_spread:.3e}", file=sys.stderr)
    return np.float32(alpha)



# revision 8
# speedup vs baseline: 1.4956x; 1.4956x over previous
"""CRF forward-algorithm (log partition) kernel for 8 Trainium2 NeuronCores.

Strategy: fp8 DoubleRow overlapped-segment exp-space scan.

The reference recurrence  fv' = logsumexp_prev(fv + T) + feat  is, in exp
space, a linear matvec chain  v' = diag(e_t) @ M @ v  with M = exp(T) fixed.
We split the T=16384 steps into S=4096 segments of L=4 and run all segments
in parallel from a guess vector, 512 columns per core, as dense
[128x(2x128)] x [128x(2x512)] fp8 DoubleRow matmuls (256-wide contraction,
2 MACs/cell/cycle — 2x bf16 PE throughput).  Products of positive matrices
contract exponentially toward rank-1 (Perron-Frobenius), so each column
only needs a short warmup to forget its guess: column j starts D=2 steps
early (at absolute step j*L-D) and the scalar mismatch at each segment
junction (kappa) is read off on the host by comparing column j's endpoint
with column j+1's warmed-up snapshot at the same absolute time.  No second
phase, no collectives.  alpha = lse(final column) + sum(kappa) + folded
scale bookkeeping, assembled on host in fp64.

Magnitudes are kept inside fp8 e4m3 range by folding a constant per-step
scale e^-sigma (sigma = log lambda1(M) + 1/2, host-estimated by power
iteration) into the psum->fp8 conversion via the scalar engine's immediate
scale; emissions stay raw exp(decoded) which already fits fp8.
"""

import numpy as np
import ml_dtypes

import concourse.bass as bass
import concourse.bacc as bacc
import concourse.mybir as mybir
import concourse.tile as tile

FP8_NP = ml_dtypes.float8_e4m3   # TRN float8e4: max 240
FP8 = mybir.dt.float8e4
BF16 = mybir.dt.bfloat16
F32 = mybir.dt.float32

SEQ_LEN = 16384
N_TAGS = 1024
START_IDX = 1022
STOP_IDX = 1023
NB = 8                  # 1024 tags = 8 planes of 128 partitions
L = 4                   # segment length (steps)
D = 2                   # guess-warmup depth (steps)
NSTEP = L + D
S = SEQ_LEN // L        # 4096 segments = columns
NCORES = 8
FD = S // NCORES        # 512 columns per core
GUESS = 1.0
BETA = 224.0            # col-0 one-hot init value

_CACHE = {}


def _build_program():
    nc = bacc.Bacc("TRN2", target_bir_lowering=False, debug=False)
    mt = nc.dram_tensor("mt", [128, NB, N_TAGS], FP8, kind="ExternalInput")
    vinit = nc.dram_tensor("vinit", [128, NB, FD], FP8, kind="ExternalInput")
    ems = nc.dram_tensor("ems", [NSTEP, 128, NB, FD], FP8, kind="ExternalInput")
    alph = nc.dram_tensor("alph", [128, 1], F32, kind="ExternalInput")
    snapd = nc.dram_tensor("snapd", [128, NB, FD], FP8, kind="ExternalOutput")
    snapl = nc.dram_tensor("snapl", [128, NB, FD], FP8, kind="ExternalOutput")
    vfin = nc.dram_tensor("vfin", [128, NB, FD], FP8, kind="ExternalOutput")

    with tile.TileContext(nc) as tc:
        with (
            tc.tile_pool(name="mpool", bufs=1) as mpool,
            tc.tile_pool(name="vpool", bufs=2) as vpool,
            tc.tile_pool(name="epool", bufs=NSTEP) as epool,
            tc.tile_pool(name="tpool", bufs=4) as tpool,
            tc.tile_pool(name="pspool", bufs=1, space="PSUM") as pspool,
        ):
            mt_sb = mpool.tile([128, NB, N_TAGS], FP8)
            nc.sync.dma_start(mt_sb[:], mt[:])
            al_sb = mpool.tile([128, 1], F32)
            nc.sync.dma_start(al_sb[:], alph[:])

            v = vpool.tile([128, NB, FD], FP8, tag="v")
            nc.sync.dma_start(v[:], vinit[:])

            for s in range(NSTEP):
                et = epool.tile([128, NB, FD], FP8, tag="em")
                nc.sync.dma_start(et[:], ems[s])
                nv = vpool.tile([128, NB, FD], FP8, tag="v")
                for mb in range(NB):
                    ps = pspool.tile([128, FD], F32, tag=f"ps{mb}")
                    for g in range(4):
                        nc.tensor.matmul(
                            ps[:],
                            mt_sb[:, 2 * g:2 * g + 2, mb * 128:(mb + 1) * 128],
                            v[:, 2 * g:2 * g + 2, :],
                            start=(g == 0),
                            stop=(g == 3),
                            perf_mode=mybir.MatmulPerfMode.DoubleRow,
                        )
                    tmp = tpool.tile([128, FD], BF16, tag="tmp")
                    nc.scalar.activation(
                        tmp[:], ps[:], mybir.ActivationFunctionType.Copy,
                        scale=al_sb[:],
                    )
                    nc.vector.tensor_mul(nv[:, mb, :], tmp[:], et[:, mb, :])
                out_d = {D: snapd, L: snapl, NSTEP: vfin}.get(s + 1)
                if out_d is not None:
                    nc.sync.dma_start(out_d[:], nv[:])
                v = nv

    nc.compile()
    return nc


def _sigma(transitions):
    """log of the dominant per-step growth: log lambda1(exp(T)) + E[log-normal
    mean of emissions] (=1/2 for N(0,1) decoded)."""
    M = np.exp(transitions.astype(np.float64))
    x = np.ones(N_TAGS)
    lam = 1.0
    for _ in range(20):
        x = M @ x
        lam = x.max()
        x /= lam
    return float(np.log(lam) + 0.5)


def _prepare_core_inputs(decoded, transitions):
    Mt = np.exp(transitions.astype(np.float32)).T        # [prev, next]
    mt = np.ascontiguousarray(
        Mt.reshape(NB, 128, N_TAGS).transpose(1, 0, 2)
    ).astype(FP8_NP)                                     # [p, kb, next]

    E8 = np.exp(decoded.astype(np.float32)).astype(FP8_NP)   # [T, N]

    # column schedule: col j>=1 local s -> t = j*L-D+s ; col 0 -> t = s
    t_of = np.empty((NSTEP, S), dtype=np.int64)
    steps = np.arange(NSTEP)
    t_of[:, 0] = steps
    t_of[:, 1:] = (np.arange(1, S) * L - D)[None, :] + steps[:, None]

    A = E8[t_of]                                         # [NSTEP, S, N]
    A = A.reshape(NSTEP, S, NB, 128).transpose(0, 3, 2, 1)   # [NSTEP, p, kb, S]

    vbase = np.full((128, NB, FD), GUESS, dtype=FP8_NP)

    sigma = _sigma(transitions)
    alph = np.full((128, 1), np.exp(-sigma), dtype=np.float32)

    in_maps = []
    for c in range(NCORES):
        ems = np.ascontiguousarray(A[:, :, :, c * FD:(c + 1) * FD])
        vin = vbase.copy()
        if c == 0:
            vin[:, :, 0] = FP8_NP(0.0)
            vin[START_IDX % 128, START_IDX // 128, 0] = FP8_NP(BETA)
        in_maps.append({"mt": mt, "vinit": vin, "ems": ems, "alph": alph})
    return in_maps, sigma


def _assemble(transitions, results, sigma):
    """Host-side kappa extraction + terminal logsumexp (tiny, fp64)."""
    def cat(key):
        # [128, NB, FD] per core -> [N_TAGS, S]
        return np.concatenate(
            [results[c][key].astype(np.float64).transpose(1, 0, 2).reshape(N_TAGS, FD)
             for c in range(NCORES)], axis=1)

    u = cat("snapd")                 # state at time j*L      (warmed guess)
    w = cat("vfin")                  # state at time (j+1)*L  (endpoint)
    z = results[0]["snapl"].astype(np.float64).transpose(1, 0, 2).reshape(N_TAGS, FD)[:, 0]

    num = np.concatenate([z[:, None], w[:, 1:S - 1]], axis=1)   # [N, S-1]
    den = u[:, 1:]
    valid = (num > 0) & (den > 0)
    with np.errstate(divide="ignore", invalid="ignore"):
        dlt = np.where(valid, np.log(num) - np.log(den), np.nan)
    kap = np.nanmedian(dlt, axis=0)                              # [S-1]

    with np.errstate(divide="ignore"):
        logx = np.log(w[:, S - 1]) + kap.sum() + SEQ_LEN * sigma - np.log(BETA)
    term = logx + transitions[STOP_IDX].astype(np.float64)
    term = term[np.isfinite(term)]
    mx = term.max()
    alpha = mx + np.log(np.exp(term - mx).sum())
    return alpha


def kernel(decoded, transitions, raw_outputs=None, outputs=None, _backend="hw"):
    decoded = np.asarray(decoded, dtype=np.float32)
    transitions = np.asarray(transitions, dtype=np.float32)

    in_maps, sigma = _prepare_core_inputs(decoded, transitions)

    if "nc" not in _CACHE:
        _CACHE["nc"] = _build_program()
    nc = _CACHE["nc"]

    if _backend == "sim":
        from concourse.bass_interp import CoreSim
        results = []
        for c in range(NCORES):
            sim = CoreSim(nc, trace=False)
            for k, v in in_maps[c].items():
                sim.tensor(k)[:] = v
            sim.simulate()
            results.append({k: np.array(sim.tensor(k))
                            for k in ("snapd", "snapl", "vfin")})
    else:
        from concourse.bass_utils import run_bass_kernel_spmd
        res = run_bass_kernel_spmd(nc, in_maps, list(range(NCORES)))
        results = res.results

    alpha = _assemble(transitions, results, sigma)
    return np.float32(alpha)
